# revision 11
# baseline (speedup 1.0000x reference)
"""DenseRelationDistill Bass/Trainium2 kernel — 8-core SPMD.

Sharding: core k handles batch b=k//2 and class-half h=k%2.
  - fa outputs: each core computes/writes 5 of the 10 classes for its batch
    (class selection is data-driven via a host-side permutation of
    `attentions`, so all 8 cores run the identical program).
  - out outputs: each core writes one 128-channel half of the 256 output
    channels for its batch (via per-core shards of the combine weights).
No collectives; host gathers per-core outputs.

All affine/BN/concat/1x1-conv algebra folded into host-prepped matrices:
  out_l = Wf @ f  +  resize16->H( A_l @ vq16 + B_l @ agg16 )  +  bias_l
  A_l = 10 * Wcomb[:,256:384] * bn_scale_l[:128]   (column-scaled)
  B_l =      Wcomb[:,384:512] * bn_scale_l[128:]
  bias_l = combine_b + Wcomb[:,256:512] @ bn_beta_l
Bilinear align_corners resizes are dense matmuls with Kronecker-product
interpolation matrices. 3x3 convs are 18 shifted-window PSUM-accumulating
matmuls with edge-clipped windows (PSUM has_written handles zero padding).
"""

import math
from contextlib import ExitStack

import numpy as np

import concourse.bass as bass
import concourse.mybir as mybir
import concourse.tile as tile
from concourse import bacc
from concourse.bass_utils import run_bass_kernel_spmd
from concourse.masks import make_identity

F32 = mybir.dt.float32
SIZES = [64, 32, 16, 8, 4]
HWS = [s * s for s in SIZES]
NCLS, KD, VD, C, BS = 10, 32, 128, 256, 4
HW16 = 256
BN_EPS = 1e-5


# ----------------------------------------------------------------- host math
def _interp_mat(n_in, n_out):
    # identical arithmetic to the reference implementation
    if n_out == 1:
        coords = np.zeros((1,))
    else:
        coords = np.arange(n_out) * (n_in - 1) / (n_out - 1)
    lo = np.clip(np.floor(coords).astype(np.int64), 0, max(n_in - 2, 0))
    frac = (coords - lo).astype(np.float32)
    M = np.zeros((n_out, n_in), dtype=np.float32)
    M[np.arange(n_out), lo] += 1.0 - frac
    if n_in > 1:
        M[np.arange(n_out), lo + 1] += frac
    return M


def _conv_w_prep(w):
    # [co, 256, 3, 3] -> [128, 3*3*2*co] sbuf layout (ci, (dy, dx, g, co))
    co = w.shape[0]
    arr = w.reshape(co, 2, 128, 3, 3).transpose(2, 3, 4, 1, 0)
    return np.ascontiguousarray(arr.reshape(128, 9 * 2 * co), dtype=np.float32)


def _ktm_prep(l):
    # downsample/upsample-to-16 matrix D = kron(Mh, Mw) [256, HW];
    # lhsT layout D^T [HW, 256]; for HW > 128, pack as [128, nkc*256]
    s = SIZES[l]
    M = _interp_mat(s, 16)
    D = np.kron(M, M).astype(np.float32)  # [256, s*s]
    Dt = np.ascontiguousarray(D.T)  # [HW, 256]
    hw = s * s
    if hw <= 128:
        return Dt
    nkc = hw // 128
    return np.ascontiguousarray(
        Dt.reshape(nkc, 128, 256).transpose(1, 0, 2).reshape(128, nkc * 256)
    )


def _rtm_prep(l):
    # upsample-from-16 matrix R = kron(M, M) [HW, 256]; rhs layout R^T packed
    # as [128, 2*HW]: rt[q, ic*HW + yx] = R^T[ic*128+q, yx]
    s = SIZES[l]
    M = _interp_mat(16, s)
    R = np.kron(M, M).astype(np.float32)  # [HW, 256]
    Rt = R.T  # [256, HW]
    hw = s * s
    return np.ascontiguousarray(
        Rt.reshape(2, 128, hw).transpose(1, 0, 2).reshape(128, 2 * hw)
    )


def _host_prep(inputs):
    """Shared (core-independent) prepped weights."""
    g = {}
    g["wkt"] = _conv_w_prep(np.asarray(inputs["key_t_w"], np.float32))
    g["wvt"] = _conv_w_prep(np.asarray(inputs["value_t_w"], np.float32))
    for l in range(5):
        g[f"wkq{l}"] = _conv_w_prep(np.asarray(inputs["key_q_w"][l], np.float32))
        g[f"wvq{l}"] = _conv_w_prep(np.asarray(inputs["value_q_w"][l], np.float32))
        if l != 2:
            g[f"ktm{l}"] = _ktm_prep(l)
            g[f"rtm{l}"] = _rtm_prep(l)
    return g


def _combine_prep(inputs, half):
    """Per-core (channel-half dependent) combine weights."""
    cw = np.asarray(inputs["combine_w"], np.float32)[:, :, 0, 0]  # [256, 512]
    cb = np.asarray(inputs["combine_b"], np.float32)  # [256]
    gam = np.asarray(inputs["bn_gamma"], np.float32)  # [5, 256]
    bet = np.asarray(inputs["bn_beta"], np.float32)  # [5, 256]
    bns = gam * np.float32(1.0 / np.sqrt(1.0 + BN_EPS))
    sl = slice(half * 128, (half + 1) * 128)
    wf = cw[sl, :256]  # [co=128, ci=256]
    wfin = cw[sl, 256:512]  # [co=128, 256]
    out = {}
    out["wf"] = np.ascontiguousarray(
        wf.reshape(128, 2, 128).transpose(2, 1, 0).reshape(128, 256)
    )
    was, wbs, bbs = [], [], []
    for l in range(5):
        A = np.float32(10.0) * wfin[:, :128] * bns[l, :128][None, :]  # [co, cv]
        B = wfin[:, 128:] * bns[l, 128:][None, :]  # [co, ca]
        was.append(np.ascontiguousarray(A.T))  # [cv, co]
        wbs.append(np.ascontiguousarray(B.T))
        bbs.append(cb[sl] + wfin @ bet[l])
    out["wa"] = np.ascontiguousarray(
        np.stack(was, 0).transpose(1, 0, 2).reshape(128, 5 * 128)
    )
    out["wb"] = np.ascontiguousarray(
        np.stack(wbs, 0).transpose(1, 0, 2).reshape(128, 5 * 128)
    )
    out["bb"] = np.ascontiguousarray(np.stack(bbs, 0).T.astype(np.float32))  # [128, 5]
    return out


# ------------------------------------------------------------ device program
def _tap_ranges(dy, dx):
    # 3x3 conv tap (dy,dx) in {0,1,2}^2 with implicit zero padding:
    # valid output rows/cols and the matching input window origin
    y0, y1 = max(0, 1 - dy), min(16, 17 - dy)
    x0, x1 = max(0, 1 - dx), min(16, 17 - dx)
    return y0, y1, x0, x1, y0 + dy - 1, x0 + dx - 1


def build_program(use_collective=True):
    nc = bacc.Bacc("TRN2", target_bir_lowering=False, num_devices=8)

    def din(name, shape):
        return nc.dram_tensor(name, list(shape), F32, kind="ExternalInput").ap()

    def dout(name, shape):
        return nc.dram_tensor(name, list(shape), F32, kind="ExternalOutput").ap()

    f_d = [din(f"f{l}", (256, HWS[l])) for l in range(5)]
    attp_d = din("attp", (5, 256, HW16))
    wkt_d = din("wkt", (128, 9 * 2 * KD))
    wvt_d = din("wvt", (128, 9 * 2 * VD))
    wkq_d = [din(f"wkq{l}", (128, 9 * 2 * KD)) for l in range(5)]
    wvq_d = [din(f"wvq{l}", (128, 9 * 2 * VD)) for l in range(5)]
    ktm_d = {}
    rtm_d = {}
    for l in (0, 1, 3, 4):
        hw = HWS[l]
        ktm_d[l] = din(f"ktm{l}", (128, (hw // 128) * 256) if hw > 128 else (hw, 256))
        rtm_d[l] = din(f"rtm{l}", (128, 2 * hw))
    wf_d = din("wf", (128, 256))
    wa_d = din("wa", (128, 5 * 128))
    wb_d = din("wb", (128, 5 * 128))
    bb_d = din("bb", (128, 5))

    fa_d = [dout(f"fa{l}", (5, HW16, HWS[l])) for l in range(5)]
    out_d = [dout(f"out{l}", (128, HWS[l])) for l in range(5)]

    cp_cnt = [0]

    def copy(dst, src):
        # alternate PSUM->SBUF copies between DVE and ACT to balance engines
        cp_cnt[0] += 1
        if cp_cnt[0] % 3 == 0:
            nc.scalar.activation(dst, src, mybir.ActivationFunctionType.Copy)
        else:
            nc.vector.tensor_copy(dst, src)

    with tile.TileContext(nc) as tc, ExitStack() as ctx:
        cpool = ctx.enter_context(tc.tile_pool(name="const", bufs=1))
        bigpool = ctx.enter_context(tc.tile_pool(name="big", bufs=1))
        wqpool = ctx.enter_context(tc.tile_pool(name="wq", bufs=1))
        sp = ctx.enter_context(tc.tile_pool(name="sp", bufs=2))
        sp4 = ctx.enter_context(tc.tile_pool(name="sp4", bufs=4))
        dram = ctx.enter_context(tc.tile_pool(name="dram", bufs=2, space="DRAM"))
        ps_hold = ctx.enter_context(tc.tile_pool(name="ph", bufs=1, space="PSUM"))
        ps_cyc = ctx.enter_context(tc.tile_pool(name="pc", bufs=7, space="PSUM"))

        _cyc_n = [0]

        def cyc(p, f):
            _cyc_n[0] += 1
            return ps_cyc.tile([p, f], F32, tag="cyc", name=f"cyc{_cyc_n[0]}")

        ident = cpool.tile([128, 128], F32, tag="ident")
        make_identity(nc, ident[:, :])

        # ---------------- setup: kt [k, j] and vtT [i, c] from attentions
        wkt = cpool.tile([128, 9 * 2 * KD], F32, tag="wkt")
        nc.sync.dma_start(wkt[:, :], wkt_d)
        wvt = cpool.tile([128, 9 * 2 * VD], F32, tag="wvt")
        nc.sync.dma_start(wvt[:, :], wvt_d)
        kt_sb = cpool.tile([KD, 5 * HW16], F32, tag="kt")
        vtT_sb = cpool.tile([128, 5 * 2 * 128], F32, tag="vtT")

        def conv3x3(ps_out, co, src_tiles, w_sb, accum=False):
            # ps_out: PSUM [co, 256]; src_tiles: two SBUF [128, 256] (ci groups)
            first, n_taps, i = (not accum), 18, 0
            for dy in range(3):
                for dx in range(3):
                    y0, y1, x0, x1, iy, ix = _tap_ranges(dy, dx)
                    ny, nx = y1 - y0, x1 - x0
                    for g in range(2):
                        src_v = src_tiles[g].rearrange("p (y x) -> p y x", y=16)
                        out_v = ps_out.rearrange("p (y x) -> p y x", y=16)
                        wofs = ((dy * 3 + dx) * 2 + g) * co
                        nc.tensor.matmul(
                            out_v[:, y0:y1, x0:x1],
                            lhsT=w_sb[:, wofs : wofs + co],
                            rhs=src_v[:, iy : iy + ny, ix : ix + nx],
                            start=(first and i == 0),
                            stop=(not accum and i == 2 * n_taps - 1),
                        )
                        i += 1

        for n in range(5):
            att = [
                sp.tile([128, HW16], F32, tag=f"att{g}", name=f"att{n}_{g}")
                for g in range(2)
            ]
            for g in range(2):
                nc.sync.dma_start(att[g][:, :], attp_d[n, g * 128 : (g + 1) * 128, :])
            ktp = cyc(KD, HW16)
            conv3x3(ktp, KD, att, wkt)
            copy(kt_sb[:, n * HW16 : (n + 1) * HW16], ktp)
            vtp = cyc(128, HW16)
            conv3x3(vtp, VD, att, wvt)
            vt_tmp = sp.tile([128, HW16], F32, tag="vt_tmp", name=f"vt_tmp{n}")
            copy(vt_tmp[:, :], vtp)
            for ic in range(2):
                tp = cyc(128, 128)
                nc.tensor.transpose(tp, vt_tmp[:, ic * 128 : (ic + 1) * 128], ident)
                copy(vtT_sb[:, (n * 2 + ic) * 128 : (n * 2 + ic + 1) * 128], tp)

        # ---------------- combine weights
        wf_sb = cpool.tile([128, 256], F32, tag="wf")
        nc.sync.dma_start(wf_sb[:, :], wf_d)
        wa_sb = cpool.tile([128, 5 * 128], F32, tag="wa")
        nc.sync.dma_start(wa_sb[:, :], wa_d)
        wb_sb = cpool.tile([128, 5 * 128], F32, tag="wb")
        nc.sync.dma_start(wb_sb[:, :], wb_d)
        bb_sb = cpool.tile([128, 5], F32, tag="bb")
        nc.sync.dma_start(bb_sb[:, :], bb_d)

        def emit_out_phase(l, hw, nyc, nfree, rtm, fr2, vq_sb, agg_out_d):
            # --- out = Wf @ f + resize(A vq + B agg) + bias
            agg_sb = sp.tile([128, HW16], F32, tag="agg_sb", name=f"agg_sb{l}", bufs=3)
            nc.sync.dma_start(agg_sb[:, :], agg_out_d[:, :])
            wa_l = wa_sb[:, l * 128 : (l + 1) * 128]
            wb_l = wb_sb[:, l * 128 : (l + 1) * 128]
            if l == 2:
                op = cyc(128, HW16)
                nc.tensor.matmul(op, lhsT=wa_l, rhs=vq_sb[:, :], start=True, stop=False)
                nc.tensor.matmul(op, lhsT=wb_l, rhs=agg_sb[:, :], start=False, stop=False)
                for g in range(2):
                    nc.tensor.matmul(
                        op,
                        lhsT=wf_sb[:, g * 128 : (g + 1) * 128],
                        rhs=fr2[g][:, :],
                        start=False,
                        stop=(g == 1),
                    )
                ob = sp4.tile([128, HW16], F32, tag="ob", name=f"ob{l}")
                nc.vector.tensor_scalar_add(ob, op, bb_sb[:, l : l + 1])
                nc.sync.dma_start(out_d[l][:, :], ob)
            else:
                # Y16^T [hw16, co] = vq^T A^T + agg^T B^T
                y16 = sp.tile([128, 256], F32, tag="y16", name=f"y16_{l}", bufs=3)
                for h in range(2):
                    yp = cyc(128, 128)
                    nc.tensor.matmul(
                        yp, lhsT=vq_sb[:, h * 128 : (h + 1) * 128], rhs=wa_l,
                        start=True, stop=False,
                    )
                    nc.tensor.matmul(
                        yp, lhsT=agg_sb[:, h * 128 : (h + 1) * 128], rhs=wb_l,
                        start=False, stop=True,
                    )
                    copy(y16[:, h * 128 : (h + 1) * 128], yp)
                for yc in range(nyc):
                    fo = []
                    for g in range(2):
                        t = sp4.tile([128, nfree], F32, tag=f"fo{g}", name=f"fo{l}_{yc}_{g}")
                        nc.sync.dma_start(
                            t[:, :],
                            f_d[l][g * 128 : (g + 1) * 128, yc * nfree : (yc + 1) * nfree],
                        )
                        fo.append(t)
                    op = cyc(128, nfree)
                    for h in range(2):
                        nc.tensor.matmul(
                            op,
                            lhsT=y16[:, h * 128 : (h + 1) * 128],
                            rhs=rtm[
                                :, h * hw + yc * nfree : h * hw + (yc + 1) * nfree
                            ],
                            start=(h == 0),
                            stop=False,
                        )
                    for g in range(2):
                        nc.tensor.matmul(
                            op,
                            lhsT=wf_sb[:, g * 128 : (g + 1) * 128],
                            rhs=fo[g][:, :],
                            start=False,
                            stop=(g == 1),
                        )
                    ob = sp4.tile([128, nfree], F32, tag="ob", name=f"ob{l}_{yc}")
                    nc.vector.tensor_scalar_add(ob, op, bb_sb[:, l : l + 1])
                    nc.sync.dma_start(
                        out_d[l][:, yc * nfree : (yc + 1) * nfree], ob
                    )

        pending_out = []

        # ---------------- per-level pipeline
        for l in range(5):
            hw = HWS[l]
            # --- fr = resize_ac(f, 16, 16), in [ci, 256] layout (2 groups)
            if l == 2:
                fr = []
                for g in range(2):
                    t = sp.tile([128, HW16], F32, tag=f"fch{g}", name=f"f2ch{g}")
                    nc.sync.dma_start(t[:, :], f_d[l][g * 128 : (g + 1) * 128, :])
                    fr.append(t)
            else:
                ktm = bigpool.tile(list(ktm_d[l].shape), F32, tag="ktm", name=f"ktm_sb{l}")
                nc.sync.dma_start(ktm[:, :], ktm_d[l])
                frT_ps = [ps_cyc.tile([128, 256], F32, tag="cyc", name=f"frT{l}_{h}") for h in range(2)]
                nkc = max(1, hw // 128)
                kp = min(128, hw)
                fch = [None, None]
                fch_base = [-1, -1]
                for kc in range(nkc):
                    for g in range(2):
                        if fch_base[g] < 0 or kc * 128 >= fch_base[g] + fch[g].shape[1]:
                            w = min(512, hw - kc * 128)
                            fch[g] = sp.tile(
                                [128, w], F32, tag=f"fch{g}", name=f"fch{l}_{g}_{kc}"
                            )
                            nc.sync.dma_start(
                                fch[g][:, :],
                                f_d[l][g * 128 : (g + 1) * 128, kc * 128 : kc * 128 + w],
                            )
                            fch_base[g] = kc * 128
                    ftc = sp.tile([kp, 256], F32, tag="ftc")
                    for g in range(2):
                        ofs = kc * 128 - fch_base[g]
                        tp = cyc(kp, 128)
                        nc.tensor.transpose(
                            tp, fch[g][:, ofs : ofs + kp], ident
                        )
                        copy(ftc[:, g * 128 : (g + 1) * 128], tp)
                    for h in range(2):
                        if hw > 128:
                            lhsT = ktm[:, kc * 256 + h * 128 : kc * 256 + h * 128 + 128]
                        else:
                            lhsT = ktm[:, h * 128 : h * 128 + 128]
                        nc.tensor.matmul(
                            frT_ps[h],
                            lhsT=lhsT,
                            rhs=ftc[:, :],
                            start=(kc == 0),
                            stop=(kc == nkc - 1),
                        )
                fr = [sp.tile([128, 256], F32, tag=f"frsb{g}", name=f"fr{l}_{g}") for g in range(2)]
                for h in range(2):
                    ft2 = sp.tile([128, 256], F32, tag="ft2")
                    copy(ft2[:, :], frT_ps[h])
                    for g in range(2):
                        tp = cyc(128, 128)
                        nc.tensor.transpose(tp, ft2[:, g * 128 : (g + 1) * 128], ident)
                        copy(fr[g][:, h * 128 : (h + 1) * 128], tp)

            # --- kq = conv3x3(fr, wkq) [32, 256]; vq = conv3x3(fr, wvq) [128, 256]
            wkq = wqpool.tile([128, 9 * 2 * KD], F32, tag="wkq")
            nc.sync.dma_start(wkq[:, :], wkq_d[l])
            wvq = wqpool.tile([128, 9 * 2 * VD], F32, tag="wvq")
            nc.sync.dma_start(wvq[:, :], wvq_d[l])
            kqp = cyc(KD, HW16)
            conv3x3(kqp, KD, fr, wkq)
            kq_sb = sp.tile([KD, HW16], F32, tag="kq")
            copy(kq_sb[:, :], kqp)
            vqp = cyc(128, HW16)
            conv3x3(vqp, VD, fr, wvq)
            vq_sb = sp.tile([128, HW16], F32, tag="vq", bufs=3)
            copy(vq_sb[:, :], vqp)

            if l != 2:
                rtm = bigpool.tile([128, 2 * hw], F32, tag=f"rtm{l % 2}", name=f"rtm_sb{l}")
                nc.sync.dma_start(rtm[:, :], rtm_d[l])

            if len(pending_out) >= 2:
                pending_out.pop(0)()

            agg_ps = ps_hold.tile([128, HW16], F32, tag="agg")
            nyc = max(1, hw // 512)
            nfree = min(hw, 512)

            # --- attention loop over classes
            for n in range(5):
                # p^T [j, i] = kt[n]^T kq ; softmax over i (free dim)
                ps_sb = []
                for jh in range(2):
                    pjh = cyc(128, HW16)
                    nc.tensor.matmul(
                        pjh,
                        lhsT=kt_sb[:, n * HW16 + jh * 128 : n * HW16 + jh * 128 + 128],
                        rhs=kq_sb[:, :],
                        start=True,
                        stop=True,
                    )
                    mx = sp.tile([128, 1], F32, tag="mx")
                    nc.vector.tensor_reduce(
                        mx, pjh, axis=mybir.AxisListType.X, op=mybir.AluOpType.max,
                        negate=True,
                    )
                    sm = sp.tile([128, 1], F32, tag="sm")
                    pse = sp.tile([128, HW16], F32, tag=f"ps{jh}")
                    nc.scalar.activation(
                        pse, pjh, mybir.ActivationFunctionType.Exp,
                        bias=mx[:, 0:1], scale=1.0, accum_out=sm[:, 0:1],
                    )
                    rs = sp.tile([128, 1], F32, tag="rs")
                    nc.vector.reciprocal(rs, sm)
                    nc.vector.tensor_scalar_mul(pse, pse, rs[:, 0:1])
                    ps_sb.append(pse)

                # pT [i, j] (transpose softmaxed p)
                pT_sb = sp4.tile([128, 512], F32, tag="pT")
                for ih in range(2):
                    for jh in range(2):
                        tp = cyc(128, 128)
                        nc.tensor.transpose(
                            tp, ps_sb[jh][:, ih * 128 : (ih + 1) * 128], ident
                        )
                        copy(
                            pT_sb[:, ih * 256 + jh * 128 : ih * 256 + jh * 128 + 128],
                            tp,
                        )

                # agg += vt[n] @ p  (accumulate across classes)
                for ic in range(2):
                    nc.tensor.matmul(
                        agg_ps,
                        lhsT=vtT_sb[:, (n * 2 + ic) * 128 : (n * 2 + ic + 1) * 128],
                        rhs=pT_sb[:, ic * 256 : (ic + 1) * 256],
                        start=(n == 0 and ic == 0),
                        stop=(n == 4 and ic == 1),
                    )

                # fa[n] = resize(p^T as [j, 16, 16] -> [j, H, W])
                if True:
                    if l == 2:
                        for jh in range(2):
                            nc.sync.dma_start(
                                fa_d[l][n, jh * 128 : (jh + 1) * 128, :], ps_sb[jh]
                            )
                    else:
                        for jh in range(2):
                            for yb in range(0, nyc, 3):
                                ycs = range(yb, min(yb + 3, nyc))
                                fps = {yc: ps_cyc.tile([128, nfree], F32, tag="cyc", name=f"fps{l}_{n}_{jh}_{yc}") for yc in ycs}
                                for ic in range(2):
                                    lh = pT_sb[
                                        :,
                                        ic * 256 + jh * 128 : ic * 256 + jh * 128 + 128,
                                    ]
                                    for yc in ycs:
                                        nc.tensor.matmul(
                                            fps[yc],
                                            lhsT=lh,
                                            rhs=rtm[
                                                :,
                                                ic * hw + yc * nfree : ic * hw
                                                + (yc + 1) * nfree,
                                            ],
                                            start=(ic == 0),
                                            stop=(ic == 1),
                                        )
                                for yc in ycs:
                                    fab = sp4.tile([128, nfree], F32, tag="fab")
                                    copy(fab[:, :], fps[yc])
                                    nc.sync.dma_start(
                                        fa_d[l][
                                            n,
                                            jh * 128 : (jh + 1) * 128,
                                            yc * nfree : (yc + 1) * nfree,
                                        ],
                                        fab,
                                    )

            # --- AllReduce partial agg (classes split across the core pair)
            agg_half = sp.tile([128, HW16], F32, tag="agg_half", bufs=3)
            copy(agg_half[:, :], agg_ps)
            agg_in_d = dram.tile([128, HW16], F32, tag="agg_in", name=f"agg_in{l}")
            agg_out_d = dram.tile([128, HW16], F32, tag="agg_out", name=f"agg_out{l}")
            nc.sync.dma_start(agg_in_d[:, :], agg_half[:, :])
            if use_collective:
                nc.gpsimd.collective_compute(
                    "AllReduce",
                    mybir.AluOpType.add,
                    replica_groups=[[0, 1], [2, 3], [4, 5], [6, 7]],
                    ins=[agg_in_d.opt()],
                    outs=[agg_out_d.opt()],
                )
            else:
                nc.sync.dma_start(agg_out_d[:, :], agg_in_d[:, :])

            def make_out_phase(l, hw, nyc, nfree, rtm, fr2, vq_sb, agg_out_d):
                def emit():
                    emit_out_phase(l, hw, nyc, nfree, rtm, fr2, vq_sb, agg_out_d)

                return emit

            pending_out.append(
                make_out_phase(
                    l, hw, nyc, nfree,
                    rtm if l != 2 else None,
                    fr if l == 2 else None,
                    vq_sb, agg_out_d,
                )
            )

        for fn_ in pending_out:
            fn_()

    nc.compile()
    return nc


# ------------------------------------------------------------------- driver
_NC_CACHE = []


def kernel(**inputs):
    inputs = {k: np.asarray(v, np.float32) for k, v in inputs.items()}
    if not _NC_CACHE:
        _NC_CACHE.append(build_program())
    nc = _NC_CACHE[0]

    shared = _host_prep(inputs)
    comb = [_combine_prep(inputs, half) for half in range(2)]
    att = inputs["attentions"].reshape(NCLS, 256, HW16)

    in_maps = []
    for k in range(8):
        b, half = k // 2, k % 2
        m = dict(shared)
        m.update(comb[half])
        m["attp"] = np.ascontiguousarray(att[half * 5 : (half + 1) * 5])
        for l in range(5):
            m[f"f{l}"] = np.ascontiguousarray(
                inputs[f"feat{l}"][b].reshape(256, HWS[l])
            )
        in_maps.append(m)

    res = run_bass_kernel_spmd(nc, in_maps, list(range(8))).results

    outs, fas = [], []
    for l in range(5):
        s = SIZES[l]
        o = np.empty((BS, 256, s, s), np.float32)
        fa = np.empty((BS, NCLS, HW16, s, s), np.float32)
        for b in range(BS):
            for half in range(2):
                r = res[2 * b + half]
                o[b, half * 128 : (half + 1) * 128] = r[f"out{l}"].reshape(128, s, s)
                fa[b, half * 5 : (half + 1) * 5] = r[f"fa{l}"].reshape(
                    5, HW16, s, s
                )
        outs.append(o)
        fas.append(fa)
    return (*outs, *fas)


# revision 12
# speedup vs baseline: 50.5463x; 50.5463x over previous
"""DenseRelationDistill Bass/Trainium2 kernel — 8-core SPMD.

Sharding: core k handles batch b=k//2 and class-half h=k%2.
  - fa outputs: each core computes/writes 5 of the 10 classes for its batch
    (class selection is data-driven via a host-side permutation of
    `attentions`, so all 8 cores run the identical program).
  - out outputs: each core writes one 128-channel half of the 256 output
    channels for its batch (via per-core shards of the combine weights).
No collectives; host gathers per-core outputs.

All affine/BN/concat/1x1-conv algebra folded into host-prepped matrices:
  out_l = Wf @ f  +  resize16->H( A_l @ vq16 + B_l @ agg16 )  +  bias_l
  A_l = 10 * Wcomb[:,256:384] * bn_scale_l[:128]   (column-scaled)
  B_l =      Wcomb[:,384:512] * bn_scale_l[128:]
  bias_l = combine_b + Wcomb[:,256:512] @ bn_beta_l
Bilinear align_corners resizes are dense matmuls with Kronecker-product
interpolation matrices. 3x3 convs are 18 shifted-window PSUM-accumulating
matmuls with edge-clipped windows (PSUM has_written handles zero padding).
"""

from contextlib import ExitStack

import numpy as np

import concourse.bass as bass
import concourse.mybir as mybir
import concourse.tile as tile
from concourse import bacc
from concourse.bass_utils import run_bass_kernel_spmd
from concourse.masks import make_identity

F32 = mybir.dt.float32
SIZES = [64, 32, 16, 8, 4]
HWS = [s * s for s in SIZES]
NCLS, KD, VD, C, BS = 10, 32, 128, 256, 4
HW16 = 256
BN_EPS = 1e-5


# ----------------------------------------------------------------- host math
def _interp_mat(n_in, n_out):
    # identical arithmetic to the reference implementation
    if n_out == 1:
        coords = np.zeros((1,))
    else:
        coords = np.arange(n_out) * (n_in - 1) / (n_out - 1)
    lo = np.clip(np.floor(coords).astype(np.int64), 0, max(n_in - 2, 0))
    frac = (coords - lo).astype(np.float32)
    M = np.zeros((n_out, n_in), dtype=np.float32)
    M[np.arange(n_out), lo] += 1.0 - frac
    if n_in > 1:
        M[np.arange(n_out), lo + 1] += frac
    return M


def _conv_w_prep(w):
    # [co, 256, 3, 3] -> [128, 3*3*2*co] sbuf layout (ci, (dy, dx, g, co))
    co = w.shape[0]
    arr = w.reshape(co, 2, 128, 3, 3).transpose(2, 3, 4, 1, 0)
    return np.ascontiguousarray(arr.reshape(128, 9 * 2 * co), dtype=np.float32)


def _ktm_prep(l):
    # downsample/upsample-to-16 matrix D = kron(Mh, Mw) [256, HW];
    # lhsT layout D^T [HW, 256]; for HW > 128, pack as [128, nkc*256]
    s = SIZES[l]
    M = _interp_mat(s, 16)
    D = np.kron(M, M).astype(np.float32)  # [256, s*s]
    Dt = np.ascontiguousarray(D.T)  # [HW, 256]
    hw = s * s
    if hw <= 128:
        return Dt
    nkc = hw // 128
    return np.ascontiguousarray(
        Dt.reshape(nkc, 128, 256).transpose(1, 0, 2).reshape(128, nkc * 256)
    )


def _rtm_prep(l):
    # upsample-from-16 matrix R = kron(M, M) [HW, 256]; rhs layout R^T packed
    # as [128, 2*HW]: rt[q, ic*HW + yx] = R^T[ic*128+q, yx]
    s = SIZES[l]
    M = _interp_mat(16, s)
    R = np.kron(M, M).astype(np.float32)  # [HW, 256]
    Rt = R.T  # [256, HW]
    hw = s * s
    return np.ascontiguousarray(
        Rt.reshape(2, 128, hw).transpose(1, 0, 2).reshape(128, 2 * hw)
    )


def _host_prep(inputs):
    """Shared (core-independent) prepped weights."""
    g = {}
    g["wkt"] = _conv_w_prep(np.asarray(inputs["key_t_w"], np.float32))
    g["wvt"] = _conv_w_prep(np.asarray(inputs["value_t_w"], np.float32))
    for l in range(5):
        g[f"wkq{l}"] = _conv_w_prep(np.asarray(inputs["key_q_w"][l], np.float32))
        g[f"wvq{l}"] = _conv_w_prep(np.asarray(inputs["value_q_w"][l], np.float32))
        if l != 2:
            g[f"ktm{l}"] = _ktm_prep(l)
            g[f"rtm{l}"] = _rtm_prep(l)
    return g


def _combine_prep(inputs, half):
    """Per-core (channel-half dependent) combine weights."""
    cw = np.asarray(inputs["combine_w"], np.float32)[:, :, 0, 0]  # [256, 512]
    cb = np.asarray(inputs["combine_b"], np.float32)  # [256]
    gam = np.asarray(inputs["bn_gamma"], np.float32)  # [5, 256]
    bet = np.asarray(inputs["bn_beta"], np.float32)  # [5, 256]
    bns = gam * np.float32(1.0 / np.sqrt(1.0 + BN_EPS))
    sl = slice(half * 128, (half + 1) * 128)
    wf = cw[sl, :256]  # [co=128, ci=256]
    wfin = cw[sl, 256:512]  # [co=128, 256]
    out = {}
    out["wf"] = np.ascontiguousarray(
        wf.reshape(128, 2, 128).transpose(2, 1, 0).reshape(128, 256)
    )
    was, wbs, bbs = [], [], []
    for l in range(5):
        A = np.float32(10.0) * wfin[:, :128] * bns[l, :128][None, :]  # [co, cv]
        B = wfin[:, 128:] * bns[l, 128:][None, :]  # [co, ca]
        was.append(np.ascontiguousarray(A.T))  # [cv, co]
        wbs.append(np.ascontiguousarray(B.T))
        bbs.append(cb[sl] + wfin @ bet[l])
    out["wa"] = np.ascontiguousarray(
        np.stack(was, 0).transpose(1, 0, 2).reshape(128, 5 * 128)
    )
    out["wb"] = np.ascontiguousarray(
        np.stack(wbs, 0).transpose(1, 0, 2).reshape(128, 5 * 128)
    )
    out["bb"] = np.ascontiguousarray(np.stack(bbs, 0).T.astype(np.float32))  # [128, 5]
    return out


# ------------------------------------------------------------ device program
def _tap_ranges(dy, dx):
    # 3x3 conv tap (dy,dx) in {0,1,2}^2 with implicit zero padding:
    # valid output rows/cols and the matching input window origin
    y0, y1 = max(0, 1 - dy), min(16, 17 - dy)
    x0, x1 = max(0, 1 - dx), min(16, 17 - dx)
    return y0, y1, x0, x1, y0 + dy - 1, x0 + dx - 1


def build_program(use_collective=True):
    nc = bacc.Bacc("TRN2", target_bir_lowering=False, num_devices=8)

    def din(name, shape):
        return nc.dram_tensor(name, list(shape), F32, kind="ExternalInput").ap()

    def dout(name, shape):
        return nc.dram_tensor(name, list(shape), F32, kind="ExternalOutput").ap()

    f_d = [din(f"f{l}", (256, HWS[l])) for l in range(5)]
    attp_d = din("attp", (5, 256, HW16))
    wkt_d = din("wkt", (128, 9 * 2 * KD))
    wvt_d = din("wvt", (128, 9 * 2 * VD))
    wkq_d = [din(f"wkq{l}", (128, 9 * 2 * KD)) for l in range(5)]
    wvq_d = [din(f"wvq{l}", (128, 9 * 2 * VD)) for l in range(5)]
    ktm_d = {}
    rtm_d = {}
    for l in (0, 1, 3, 4):
        hw = HWS[l]
        ktm_d[l] = din(f"ktm{l}", (128, (hw // 128) * 256) if hw > 128 else (hw, 256))
        rtm_d[l] = din(f"rtm{l}", (128, 2 * hw))
    wf_d = din("wf", (128, 256))
    wa_d = din("wa", (128, 5 * 128))
    wb_d = din("wb", (128, 5 * 128))
    bb_d = din("bb", (128, 5))

    fa_d = [dout(f"fa{l}", (5, HW16, HWS[l])) for l in range(5)]
    out_d = [dout(f"out{l}", (128, HWS[l])) for l in range(5)]

    cp_cnt = [0]

    def copy(dst, src):
        # alternate PSUM->SBUF copies between DVE and ACT to balance engines
        cp_cnt[0] += 1
        if cp_cnt[0] % 3 == 0:
            nc.scalar.activation(dst, src, mybir.ActivationFunctionType.Copy)
        else:
            nc.vector.tensor_copy(dst, src)

    with tile.TileContext(nc) as tc, ExitStack() as ctx:
        cpool = ctx.enter_context(tc.tile_pool(name="const", bufs=1))
        bigpool = ctx.enter_context(tc.tile_pool(name="big", bufs=1))
        wqpool = ctx.enter_context(tc.tile_pool(name="wq", bufs=1))
        sp = ctx.enter_context(tc.tile_pool(name="sp", bufs=2))
        sp4 = ctx.enter_context(tc.tile_pool(name="sp4", bufs=4))
        dram = ctx.enter_context(tc.tile_pool(name="dram", bufs=2, space="DRAM"))
        ps_hold = ctx.enter_context(tc.tile_pool(name="ph", bufs=1, space="PSUM"))
        ps_cyc = ctx.enter_context(tc.tile_pool(name="pc", bufs=7, space="PSUM"))

        _cyc_n = [0]

        def cyc(p, f):
            _cyc_n[0] += 1
            return ps_cyc.tile([p, f], F32, tag="cyc", name=f"cyc{_cyc_n[0]}")

        ident = cpool.tile([128, 128], F32, tag="ident")
        make_identity(nc, ident[:, :])

        # ---------------- setup: kt [k, j] and vtT [i, c] from attentions
        wkt = cpool.tile([128, 9 * 2 * KD], F32, tag="wkt")
        nc.sync.dma_start(wkt[:, :], wkt_d)
        wvt = cpool.tile([128, 9 * 2 * VD], F32, tag="wvt")
        nc.sync.dma_start(wvt[:, :], wvt_d)
        kt_sb = cpool.tile([KD, 5 * HW16], F32, tag="kt")
        vtT_sb = cpool.tile([128, 5 * 2 * 128], F32, tag="vtT")

        def conv3x3(ps_out, co, src_tiles, w_sb, accum=False):
            # ps_out: PSUM [co, 256]; src_tiles: two SBUF [128, 256] (ci groups)
            first, n_taps, i = (not accum), 18, 0
            for dy in range(3):
                for dx in range(3):
                    y0, y1, x0, x1, iy, ix = _tap_ranges(dy, dx)
                    ny, nx = y1 - y0, x1 - x0
                    for g in range(2):
                        src_v = src_tiles[g].rearrange("p (y x) -> p y x", y=16)
                        out_v = ps_out.rearrange("p (y x) -> p y x", y=16)
                        wofs = ((dy * 3 + dx) * 2 + g) * co
                        nc.tensor.matmul(
                            out_v[:, y0:y1, x0:x1],
                            lhsT=w_sb[:, wofs : wofs + co],
                            rhs=src_v[:, iy : iy + ny, ix : ix + nx],
                            start=(first and i == 0),
                            stop=(not accum and i == 2 * n_taps - 1),
                        )
                        i += 1

        for n in range(5):
            att = [
                sp.tile([128, HW16], F32, tag=f"att{g}", name=f"att{n}_{g}")
                for g in range(2)
            ]
            for g in range(2):
                nc.sync.dma_start(att[g][:, :], attp_d[n, g * 128 : (g + 1) * 128, :])
            ktp = cyc(KD, HW16)
            conv3x3(ktp, KD, att, wkt)
            copy(kt_sb[:, n * HW16 : (n + 1) * HW16], ktp)
            vtp = cyc(128, HW16)
            conv3x3(vtp, VD, att, wvt)
            vt_tmp = sp.tile([128, HW16], F32, tag="vt_tmp", name=f"vt_tmp{n}")
            copy(vt_tmp[:, :], vtp)
            for ic in range(2):
                tp = cyc(128, 128)
                nc.tensor.transpose(tp, vt_tmp[:, ic * 128 : (ic + 1) * 128], ident)
                copy(vtT_sb[:, (n * 2 + ic) * 128 : (n * 2 + ic + 1) * 128], tp)

        # ---------------- combine weights
        wf_sb = cpool.tile([128, 256], F32, tag="wf")
        nc.sync.dma_start(wf_sb[:, :], wf_d)
        wa_sb = cpool.tile([128, 5 * 128], F32, tag="wa")
        nc.sync.dma_start(wa_sb[:, :], wa_d)
        wb_sb = cpool.tile([128, 5 * 128], F32, tag="wb")
        nc.sync.dma_start(wb_sb[:, :], wb_d)
        bb_sb = cpool.tile([128, 5], F32, tag="bb")
        nc.sync.dma_start(bb_sb[:, :], bb_d)

        def emit_out_phase(l, hw, nyc, nfree, rtm, fr2, vq_sb, agg_out_d):
            # --- out = Wf @ f + resize(A vq + B agg) + bias
            agg_sb = sp.tile([128, HW16], F32, tag="agg_sb", name=f"agg_sb{l}", bufs=3)
            nc.sync.dma_start(agg_sb[:, :], agg_out_d[:, :])
            wa_l = wa_sb[:, l * 128 : (l + 1) * 128]
            wb_l = wb_sb[:, l * 128 : (l + 1) * 128]
            if l == 2:
                op = cyc(128, HW16)
                nc.tensor.matmul(op, lhsT=wa_l, rhs=vq_sb[:, :], start=True, stop=False)
                nc.tensor.matmul(op, lhsT=wb_l, rhs=agg_sb[:, :], start=False, stop=False)
                for g in range(2):
                    nc.tensor.matmul(
                        op,
                        lhsT=wf_sb[:, g * 128 : (g + 1) * 128],
                        rhs=fr2[g][:, :],
                        start=False,
                        stop=(g == 1),
                    )
                ob = sp4.tile([128, HW16], F32, tag="ob", name=f"ob{l}")
                nc.vector.tensor_scalar_add(ob, op, bb_sb[:, l : l + 1])
                nc.sync.dma_start(out_d[l][:, :], ob)
            else:
                # Y16^T [hw16, co] = vq^T A^T + agg^T B^T
                y16 = sp.tile([128, 256], F32, tag="y16", name=f"y16_{l}", bufs=3)
                for h in range(2):
                    yp = cyc(128, 128)
                    nc.tensor.matmul(
                        yp, lhsT=vq_sb[:, h * 128 : (h + 1) * 128], rhs=wa_l,
                        start=True, stop=False,
                    )
                    nc.tensor.matmul(
                        yp, lhsT=agg_sb[:, h * 128 : (h + 1) * 128], rhs=wb_l,
                        start=False, stop=True,
                    )
                    copy(y16[:, h * 128 : (h + 1) * 128], yp)
                for yc in range(nyc):
                    fo = []
                    for g in range(2):
                        t = sp4.tile([128, nfree], F32, tag=f"fo{g}", name=f"fo{l}_{yc}_{g}")
                        nc.sync.dma_start(
                            t[:, :],
                            f_d[l][g * 128 : (g + 1) * 128, yc * nfree : (yc + 1) * nfree],
                        )
                        fo.append(t)
                    op = cyc(128, nfree)
                    for h in range(2):
                        nc.tensor.matmul(
                            op,
                            lhsT=y16[:, h * 128 : (h + 1) * 128],
                            rhs=rtm[
                                :, h * hw + yc * nfree : h * hw + (yc + 1) * nfree
                            ],
                            start=(h == 0),
                            stop=False,
                        )
                    for g in range(2):
                        nc.tensor.matmul(
                            op,
                            lhsT=wf_sb[:, g * 128 : (g + 1) * 128],
                            rhs=fo[g][:, :],
                            start=False,
                            stop=(g == 1),
                        )
                    ob = sp4.tile([128, nfree], F32, tag="ob", name=f"ob{l}_{yc}")
                    nc.vector.tensor_scalar_add(ob, op, bb_sb[:, l : l + 1])
                    nc.sync.dma_start(
                        out_d[l][:, yc * nfree : (yc + 1) * nfree], ob
                    )

        pending_out = []

        # ---------------- per-level pipeline
        for l in range(5):
            hw = HWS[l]
            # --- fr = resize_ac(f, 16, 16), in [ci, 256] layout (2 groups)
            if l == 2:
                fr = []
                for g in range(2):
                    t = sp.tile([128, HW16], F32, tag=f"fch{g}", name=f"f2ch{g}")
                    nc.sync.dma_start(t[:, :], f_d[l][g * 128 : (g + 1) * 128, :])
                    fr.append(t)
            else:
                ktm = bigpool.tile(list(ktm_d[l].shape), F32, tag="ktm", name=f"ktm_sb{l}")
                nc.sync.dma_start(ktm[:, :], ktm_d[l])
                frT_ps = [ps_cyc.tile([128, 256], F32, tag="cyc", name=f"frT{l}_{h}") for h in range(2)]
                nkc = max(1, hw // 128)
                kp = min(128, hw)
                fch = [None, None]
                fch_base = [-1, -1]
                for kc in range(nkc):
                    for g in range(2):
                        if fch_base[g] < 0 or kc * 128 >= fch_base[g] + fch[g].shape[1]:
                            w = min(512, hw - kc * 128)
                            fch[g] = sp.tile(
                                [128, w], F32, tag=f"fch{g}", name=f"fch{l}_{g}_{kc}"
                            )
                            nc.sync.dma_start(
                                fch[g][:, :],
                                f_d[l][g * 128 : (g + 1) * 128, kc * 128 : kc * 128 + w],
                            )
                            fch_base[g] = kc * 128
                    ftc = sp.tile([kp, 256], F32, tag="ftc")
                    for g in range(2):
                        ofs = kc * 128 - fch_base[g]
                        tp = cyc(kp, 128)
                        nc.tensor.transpose(
                            tp, fch[g][:, ofs : ofs + kp], ident
                        )
                        copy(ftc[:, g * 128 : (g + 1) * 128], tp)
                    for h in range(2):
                        if hw > 128:
                            lhsT = ktm[:, kc * 256 + h * 128 : kc * 256 + h * 128 + 128]
                        else:
                            lhsT = ktm[:, h * 128 : h * 128 + 128]
                        nc.tensor.matmul(
                            frT_ps[h],
                            lhsT=lhsT,
                            rhs=ftc[:, :],
                            start=(kc == 0),
                            stop=(kc == nkc - 1),
                        )
                fr = [sp.tile([128, 256], F32, tag=f"frsb{g}", name=f"fr{l}_{g}") for g in range(2)]
                for h in range(2):
                    ft2 = sp.tile([128, 256], F32, tag="ft2")
                    copy(ft2[:, :], frT_ps[h])
                    for g in range(2):
                        tp = cyc(128, 128)
                        nc.tensor.transpose(tp, ft2[:, g * 128 : (g + 1) * 128], ident)
                        copy(fr[g][:, h * 128 : (h + 1) * 128], tp)

            # --- kq = conv3x3(fr, wkq) [32, 256]; vq = conv3x3(fr, wvq) [128, 256]
            wkq = wqpool.tile([128, 9 * 2 * KD], F32, tag="wkq")
            nc.sync.dma_start(wkq[:, :], wkq_d[l])
            wvq = wqpool.tile([128, 9 * 2 * VD], F32, tag="wvq")
            nc.sync.dma_start(wvq[:, :], wvq_d[l])
            kqp = cyc(KD, HW16)
            conv3x3(kqp, KD, fr, wkq)
            kq_sb = sp.tile([KD, HW16], F32, tag="kq")
            copy(kq_sb[:, :], kqp)
            vqp = cyc(128, HW16)
            conv3x3(vqp, VD, fr, wvq)
            vq_sb = sp.tile([128, HW16], F32, tag="vq", bufs=3)
            copy(vq_sb[:, :], vqp)

            if l != 2:
                rtm = bigpool.tile([128, 2 * hw], F32, tag=f"rtm{l % 2}", name=f"rtm_sb{l}")
                nc.sync.dma_start(rtm[:, :], rtm_d[l])

            if len(pending_out) >= 2:
                pending_out.pop(0)()

            agg_ps = ps_hold.tile([128, HW16], F32, tag="agg")
            nyc = max(1, hw // 512)
            nfree = min(hw, 512)

            # --- attention loop over classes
            for n in range(5):
                # p^T [j, i] = kt[n]^T kq ; softmax over i (free dim)
                ps_sb = []
                for jh in range(2):
                    pjh = cyc(128, HW16)
                    nc.tensor.matmul(
                        pjh,
                        lhsT=kt_sb[:, n * HW16 + jh * 128 : n * HW16 + jh * 128 + 128],
                        rhs=kq_sb[:, :],
                        start=True,
                        stop=True,
                    )
                    mx = sp.tile([128, 1], F32, tag="mx")
                    nc.vector.tensor_reduce(
                        mx, pjh, axis=mybir.AxisListType.X, op=mybir.AluOpType.max,
                        negate=True,
                    )
                    sm = sp.tile([128, 1], F32, tag="sm")
                    pse = sp.tile([128, HW16], F32, tag=f"ps{jh}")
                    nc.scalar.activation(
                        pse, pjh, mybir.ActivationFunctionType.Exp,
                        bias=mx[:, 0:1], scale=1.0, accum_out=sm[:, 0:1],
                    )
                    rs = sp.tile([128, 1], F32, tag="rs")
                    nc.vector.reciprocal(rs, sm)
                    nc.vector.tensor_scalar_mul(pse, pse, rs[:, 0:1])
                    ps_sb.append(pse)

                # pT [i, j] (transpose softmaxed p)
                pT_sb = sp4.tile([128, 512], F32, tag="pT")
                for ih in range(2):
                    for jh in range(2):
                        tp = cyc(128, 128)
                        nc.tensor.transpose(
                            tp, ps_sb[jh][:, ih * 128 : (ih + 1) * 128], ident
                        )
                        copy(
                            pT_sb[:, ih * 256 + jh * 128 : ih * 256 + jh * 128 + 128],
                            tp,
                        )

                # agg += vt[n] @ p  (accumulate across classes)
                for ic in range(2):
                    nc.tensor.matmul(
                        agg_ps,
                        lhsT=vtT_sb[:, (n * 2 + ic) * 128 : (n * 2 + ic + 1) * 128],
                        rhs=pT_sb[:, ic * 256 : (ic + 1) * 256],
                        start=(n == 0 and ic == 0),
                        stop=(n == 4 and ic == 1),
                    )

                # fa[n] = resize(p^T as [j, 16, 16] -> [j, H, W])
                if True:
                    if l == 2:
                        for jh in range(2):
                            nc.sync.dma_start(
                                fa_d[l][n, jh * 128 : (jh + 1) * 128, :], ps_sb[jh]
                            )
                    else:
                        for jh in range(2):
                            for yb in range(0, nyc, 3):
                                ycs = range(yb, min(yb + 3, nyc))
                                fps = {yc: ps_cyc.tile([128, nfree], F32, tag="cyc", name=f"fps{l}_{n}_{jh}_{yc}") for yc in ycs}
                                for ic in range(2):
                                    lh = pT_sb[
                                        :,
                                        ic * 256 + jh * 128 : ic * 256 + jh * 128 + 128,
                                    ]
                                    for yc in ycs:
                                        nc.tensor.matmul(
                                            fps[yc],
                                            lhsT=lh,
                                            rhs=rtm[
                                                :,
                                                ic * hw + yc * nfree : ic * hw
                                                + (yc + 1) * nfree,
                                            ],
                                            start=(ic == 0),
                                            stop=(ic == 1),
                                        )
                                for yc in ycs:
                                    fab = sp4.tile([128, nfree], F32, tag="fab")
                                    copy(fab[:, :], fps[yc])
                                    nc.sync.dma_start(
                                        fa_d[l][
                                            n,
                                            jh * 128 : (jh + 1) * 128,
                                            yc * nfree : (yc + 1) * nfree,
                                        ],
                                        fab,
                                    )

            # --- AllReduce partial agg (classes split across the core pair)
            agg_half = sp.tile([128, HW16], F32, tag="agg_half", bufs=3)
            copy(agg_half[:, :], agg_ps)
            agg_in_d = dram.tile([128, HW16], F32, tag="agg_in", name=f"agg_in{l}")
            agg_out_d = dram.tile([128, HW16], F32, tag="agg_out", name=f"agg_out{l}")
            nc.sync.dma_start(agg_in_d[:, :], agg_half[:, :])
            if use_collective:
                nc.gpsimd.collective_compute(
                    "AllReduce",
                    mybir.AluOpType.add,
                    replica_groups=[[0, 1], [2, 3], [4, 5], [6, 7]],
                    ins=[agg_in_d.opt()],
                    outs=[agg_out_d.opt()],
                )
            else:
                nc.sync.dma_start(agg_out_d[:, :], agg_in_d[:, :])

            def make_out_phase(l, hw, nyc, nfree, rtm, fr2, vq_sb, agg_out_d):
                def emit():
                    emit_out_phase(l, hw, nyc, nfree, rtm, fr2, vq_sb, agg_out_d)

                return emit

            pending_out.append(
                make_out_phase(
                    l, hw, nyc, nfree,
                    rtm if l != 2 else None,
                    fr if l == 2 else None,
                    vq_sb, agg_out_d,
                )
            )

        for fn_ in pending_out:
            fn_()

    nc.compile()
    return nc


# ------------------------------------------------------------------- driver
_NC_CACHE = []


def kernel(**inputs):
    inputs = {k: np.asarray(v, np.float32) for k, v in inputs.items()}
    if not _NC_CACHE:
        _NC_CACHE.append(build_program())
    nc = _NC_CACHE[0]

    shared = _host_prep(inputs)
    comb = [_combine_prep(inputs, half) for half in range(2)]
    att = inputs["attentions"].reshape(NCLS, 256, HW16)

    in_maps = []
    for k in range(8):
        b, half = k // 2, k % 2
        m = dict(shared)
        m.update(comb[half])
        m["attp"] = np.ascontiguousarray(att[half * 5 : (half + 1) * 5])
        for l in range(5):
            m[f"f{l}"] = np.ascontiguousarray(
                inputs[f"feat{l}"][b].reshape(256, HWS[l])
            )
        in_maps.append(m)

    res = run_bass_kernel_spmd(nc, in_maps, list(range(8))).results

    outs, fas = [], []
    for l in range(5):
        s = SIZES[l]
        o = np.empty((BS, 256, s, s), np.float32)
        fa = np.empty((BS, NCLS, HW16, s, s), np.float32)
        for b in range(BS):
            for half in range(2):
                r = res[2 * b + half]
                o[b, half * 128 : (half + 1) * 128] = r[f"out{l}"].reshape(128, s, s)
                fa[b, half * 5 : (half + 1) * 5] = r[f"fa{l}"].reshape(
                    5, HW16, s, s
                )
        outs.append(o)
        fas.append(fa)
    return (*outs, *fas)
